# revision 4
# baseline (speedup 1.0000x reference)
"""Trainium2 Bass kernel v5: v4 + packed gather calls.

Per chunk, cells (tile, chunk) are concatenated at 16-row granularity and
sliced into <=1024-row dma_gather calls (~53 per chunk, ~212 per layer vs 392),
amortizing the ~1us fixed SWDGE descriptor-generation cost per call. A call's
128-slot blocks may span two cells (tiles); each (block, cell) intersection is
a "piece" with its own masked one-hot column, matmul-accumulated into the
owning tile's PSUM.
"""

import numpy as np
import ml_dtypes

P = 128
NCORES = 8
N_NODES = 100000
NLOC_REAL = 12500
NT = 98
NLOC = NT * P
QL = NLOC // 4                 # 3136 local rows per quarter
CR = NCORES * QL               # 25088 rows per quarter table (< 32768)
NCHUNK = 4
IN_C = 128
HID = 64
TABW = 128
CALL_ROWS = 1024
BMAX = CALL_ROWS // P          # 8 blocks per call
OHMAX = 12                     # pieces per call (<= blocks + cells-1)


def _prep(x, edge_index, W1, b1, W2, b2, Wl, bl):
    src = np.asarray(edge_index[0], dtype=np.int64)
    dst = np.asarray(edge_index[1], dtype=np.int64)
    core = dst // NLOC_REAL
    dst_local = dst - core * NLOC_REAL
    sk = src // NLOC_REAL
    si = src - sk * NLOC_REAL
    chunk = si // QL
    src_rel = sk * QL + (si - chunk * QL)
    tile = dst_local // P

    ncell = NT * NCHUNK
    cell = core * ncell + tile * NCHUNK + chunk
    order = np.argsort(cell, kind="stable")
    counts = np.bincount(cell, minlength=NCORES * ncell)
    src_s = src_rel[order].astype(np.int16)
    dstl_s = (dst_local[order] - tile[order] * P).astype(np.float32)
    cell_start = np.zeros(NCORES * ncell + 1, dtype=np.int64)
    np.cumsum(counts, out=cell_start[1:])

    cnt_ctc = counts.reshape(NCORES, NT, NCHUNK)
    r16_tc = np.maximum(16, -(-cnt_ctc.max(axis=0) // 16) * 16)   # [NT, NCHUNK]

    # chunk streams: cell (t, c) occupies stream-c rows [s0[t,c], s0[t,c]+r16)
    s0 = np.zeros((NT, NCHUNK), dtype=np.int64)
    for c in range(NCHUNK):
        s0[1:, c] = np.cumsum(r16_tc[:-1, c])
    stream_len = [int(s0[-1, c] + r16_tc[-1, c]) for c in range(NCHUNK)]

    # calls: slices of each stream, <= CALL_ROWS, aligned to CALL_ROWS grid
    calls = []          # (chunk, s_begin, rows, col_off)
    col_off = 0
    call_of = {}        # (chunk, call_idx_in_chunk) -> global call id
    for c in range(NCHUNK):
        nci = -(-stream_len[c] // CALL_ROWS)
        for j in range(nci):
            a = j * CALL_ROWS
            b = min(stream_len[c], a + CALL_ROWS)
            call_of[(c, j)] = len(calls)
            calls.append((c, a, b - a, col_off))
            col_off += (b - a) // 16
    gcols = col_off

    # pieces: per call, per 128-block, intersections with cells
    # piece: (call_id, blk_in_call, tile, ohcol, lo_in_cell, hi_in_cell, lo_in_blk)
    pieces_by_tile = [[] for _ in range(NT)]
    npieces = 0
    call_pieces = [[] for _ in calls]   # ohcols per call in order
    for cid, (c, a, rows, coff) in enumerate(calls):
        nblk = -(-rows // P)
        for b in range(nblk):
            blo = a + b * P
            bhi = min(a + rows, blo + P)
            # find cells overlapping [blo, bhi) in stream c
            t_lo = int(np.searchsorted(s0[:, c], blo, side="right")) - 1
            t_hi = int(np.searchsorted(s0[:, c], bhi - 1, side="right")) - 1
            for t in range(t_lo, t_hi + 1):
                clo, chi = int(s0[t, c]), int(s0[t, c] + r16_tc[t, c])
                lo = max(blo, clo)
                hi = min(bhi, chi)
                if lo >= hi:
                    continue
                ohcol = npieces
                npieces += 1
                call_pieces[cid].append(ohcol)
                pieces_by_tile[t].append(
                    (cid, b, ohcol, lo - clo, hi - clo, lo - blo))
    assert max(len(cp) for cp in call_pieces) <= OHMAX

    meta = {"calls": calls, "call_pieces": call_pieces,
            "pieces_by_tile": pieces_by_tile, "npieces": npieces,
            "gcols": gcols}

    in_maps = []
    for cc in range(NCORES):
        m = {}
        xl = np.zeros((P, NLOC), dtype=np.float32)
        xl[:, :NLOC_REAL] = np.asarray(
            x[cc * NLOC_REAL:(cc + 1) * NLOC_REAL], np.float32).T
        m["xT"] = np.ascontiguousarray(xl)

        deg = np.bincount(dst_local[core == cc], minlength=NLOC).astype(np.float64)
        deg += 1.0
        m["dis"] = np.ascontiguousarray(
            (1.0 / np.sqrt(deg)).astype(np.float32).reshape(NT, P).T)

        # per-core idx stream per chunk (cells packed, cnt real + pad 0)
        streams = []
        dstv = []        # per-cell dstrel values (cnt real, pad -1 to r16)
        for c in range(NCHUNK):
            s = np.zeros(stream_len[c], dtype=np.int16)
            streams.append(s)
        cellv = {}
        for t in range(NT):
            for c in range(NCHUNK):
                cid2 = cc * ncell + t * NCHUNK + c
                lo, hi = cell_start[cid2], cell_start[cid2 + 1]
                cnt = int(hi - lo)
                r16 = int(r16_tc[t, c])
                a = int(s0[t, c])
                streams[c][a:a + cnt] = src_s[lo:hi]
                dv = np.full(r16, -1.0, dtype=np.float32)
                dv[:cnt] = dstl_s[lo:hi]
                cellv[(t, c)] = dv
        gidx = np.zeros((P, gcols), dtype=np.int16)
        for cid, (c, a, rows, coff) in enumerate(calls):
            seg = streams[c][a:a + rows]
            w = seg.reshape(-1, 16).T
            gidx[:, coff:coff + rows // 16] = np.tile(w, (8, 1))
        m["gidx"] = np.ascontiguousarray(gidx)

        dstrel = np.full((P, npieces), -1.0, dtype=np.float32)
        for t in range(NT):
            for (cid, b, ohcol, lo_c, hi_c, lo_b) in pieces_by_tile[t]:
                c = calls[cid][0]
                dv = cellv[(t, c)]
                n = hi_c - lo_c
                dstrel[lo_b:lo_b + n, ohcol] = dv[lo_c:hi_c]
        m["dstrel"] = np.ascontiguousarray(dstrel.astype(ml_dtypes.bfloat16))

        m["identm"] = np.eye(P, dtype=np.float32)
        m["iota"] = np.ascontiguousarray(
            np.tile(np.arange(P, dtype=np.float32), (P, 1)).astype(
                ml_dtypes.bfloat16))
        m["W1"] = np.asarray(W1, np.float32)
        m["W2"] = np.asarray(W2, np.float32)
        m["b1b"] = np.ascontiguousarray(
            np.tile(np.asarray(b1, np.float32)[None, :], (P, 1)))
        m["b2b"] = np.ascontiguousarray(
            np.tile(np.asarray(b2, np.float32)[None, :], (P, 1)))
        m["Wlb"] = np.ascontiguousarray(
            np.tile(np.asarray(Wl, np.float32)[:, 0][None, :], (P, 1)))
        in_maps.append(m)
    return in_maps, meta


def _program(meta, bl_value, linearize=False):
    from concourse import bass, bacc, mybir
    import concourse.tile as tile

    f32 = mybir.dt.float32
    bf16 = mybir.dt.bfloat16
    i16 = mybir.dt.int16
    AF = mybir.ActivationFunctionType
    OP = mybir.AluOpType

    calls = meta["calls"]
    call_pieces = meta["call_pieces"]
    pieces_by_tile = meta["pieces_by_tile"]
    NPIECES, GCOLS = meta["npieces"], meta["gcols"]

    nc = bacc.Bacc("TRN2", target_bir_lowering=False, debug=False,
                   num_devices=NCORES, num_swdge_queues=4)
    xT_d = nc.dram_tensor("xT", [P, NLOC], f32, kind="ExternalInput")
    dis_d = nc.dram_tensor("dis", [P, NT], f32, kind="ExternalInput")
    gidx_d = nc.dram_tensor("gidx", [P, GCOLS], i16, kind="ExternalInput")
    dstrel_d = nc.dram_tensor("dstrel", [P, NPIECES], bf16, kind="ExternalInput")
    iota_d = nc.dram_tensor("iota", [P, P], bf16, kind="ExternalInput")
    identm_d = nc.dram_tensor("identm", [P, P], f32, kind="ExternalInput")
    W1_d = nc.dram_tensor("W1", [IN_C, HID], f32, kind="ExternalInput")
    W2_d = nc.dram_tensor("W2", [HID, HID], f32, kind="ExternalInput")
    b1b_d = nc.dram_tensor("b1b", [P, HID], f32, kind="ExternalInput")
    b2b_d = nc.dram_tensor("b2b", [P, HID], f32, kind="ExternalInput")
    Wlb_d = nc.dram_tensor("Wlb", [P, HID], f32, kind="ExternalInput")
    out_d = nc.dram_tensor("out", [NT, P], f32, kind="ExternalOutput")

    hlocq = [[nc.dram_tensor(f"h{l}q{q}", [QL, TABW], bf16) for q in range(4)]
             for l in (1, 2)]
    tabq = [[nc.dram_tensor(f"tab{l}q{q}", [CR, TABW], bf16,
                            addr_space="Shared") for q in range(4)]
            for l in (1, 2)]
    rg = [list(range(NCORES))]

    with tile.TileContext(nc, linearize=linearize) as tc:
        from contextlib import ExitStack
        with ExitStack() as ctx:
            const = ctx.enter_context(tc.tile_pool(name="const", bufs=1))
            persist = ctx.enter_context(tc.tile_pool(name="persist", bufs=1))
            tmp = ctx.enter_context(tc.tile_pool(name="tmp", bufs=6))
            psum = ctx.enter_context(tc.tile_pool(name="psum", bufs=3,
                                                  space="PSUM"))
            psumT = ctx.enter_context(tc.tile_pool(name="psumT", bufs=1,
                                                   space="PSUM"))

            ident = const.tile([P, P], f32, tag="ident")
            nc.sync.dma_start(out=ident[:], in_=identm_d[:, :])
            identb = const.tile([P, P], bf16, tag="identb")
            nc.vector.tensor_copy(identb[:], ident[:])
            iota_t = const.tile([P, P], bf16, tag="iota")
            nc.sync.dma_start(out=iota_t[:], in_=iota_d[:, :])
            W1_t = const.tile([IN_C, HID], f32, tag="W1")
            nc.sync.dma_start(out=W1_t[:], in_=W1_d[:, :])
            W2_t = const.tile([HID, HID], f32, tag="W2")
            nc.sync.dma_start(out=W2_t[:], in_=W2_d[:, :])
            b1_t = const.tile([P, HID], f32, tag="b1")
            nc.sync.dma_start(out=b1_t[:], in_=b1b_d[:, :])
            b2_t = const.tile([P, HID], f32, tag="b2")
            nc.sync.dma_start(out=b2_t[:], in_=b2b_d[:, :])
            Wl_t = const.tile([P, HID], f32, tag="Wl")
            nc.sync.dma_start(out=Wl_t[:], in_=Wlb_d[:, :])
            bl_t = const.tile([P, 1], f32, tag="bl")
            nc.vector.memset(bl_t[:], float(bl_value))
            dis_t = const.tile([P, NT], f32, tag="dis")
            nc.sync.dma_start(out=dis_t[:], in_=dis_d[:, :])
            dstrel_t = const.tile([P, NPIECES], bf16, tag="dstrel")
            nc.sync.dma_start(out=dstrel_t[:], in_=dstrel_d[:, :])
            gidx_t = const.tile([P, GCOLS], i16, tag="gidx")
            nc.sync.dma_start(out=gidx_t[:], in_=gidx_d[:, :])

            hp_sb1 = persist.tile([P, NT * HID], bf16, tag="hp_sb1")
            hp_sb2 = persist.tile([P, NT * HID], bf16, tag="hp_sb2")
            hp_sb = [hp_sb1, hp_sb2]
            zT_sb = persist.tile([HID, NT * P], f32, tag="zT_sb")
            y_sb = persist.tile([P, NT], f32, tag="y_sb")

            nreg = {}

            def reg_for(n):
                if n not in nreg:
                    nreg[n] = nc.gpsimd.to_reg(n)
                return nreg[n]

            def tile_A(l, t, xT_t):
                W_t = W1_t if l == 1 else W2_t
                ps = psum.tile([P, HID], f32, tag="psA")
                if l == 1:
                    lhsT = xT_t[:, t * P:(t + 1) * P]
                else:
                    lhsT = zT_sb[:, t * P:(t + 1) * P]
                nc.tensor.matmul(out=ps[:], lhsT=lhsT, rhs=W_t[:],
                                 start=True, stop=True)
                hp = hp_sb[l - 1][:, t * HID:(t + 1) * HID]
                nc.scalar.activation(out=hp, in_=ps[:], func=AF.Copy,
                                     scale=dis_t[:, t:t + 1])
                r0, r1 = t * P, (t + 1) * P
                q0, q1 = r0 // QL, (r1 - 1) // QL
                for q in range(q0, q1 + 1):
                    a = max(r0, q * QL)
                    b = min(r1, (q + 1) * QL)
                    nc.sync.dma_start(
                        out=hlocq[l - 1][q][a - q * QL:b - q * QL, 0:HID],
                        in_=hp[a - r0:b - r0, :])

            def fire_AG(l, q):
                nc.gpsimd.collective_compute(
                    "AllGather", mybir.AluOpType.bypass, replica_groups=rg,
                    ins=[hlocq[l - 1][q][:, :]], outs=[tabq[l - 1][q][:, :]])

            q_last_tile = [-(-(q + 1) * QL // P) - 1 for q in range(4)]
            qn_state = [0]

            def layer_B(l, gp, ohp, interleave_A2):
                """emit calls lazily in tile order; piece-matmuls per tile."""
                b_t = b1_t if l == 1 else b2_t
                gf_of = {}
                oh_of = {}

                def emit_call(cid):
                    (c, a, rows, coff) = calls[cid]
                    nblk = -(-rows // P)
                    # one-hot for all pieces of this call
                    pcs = call_pieces[cid]
                    oh = ohp.tile([P, OHMAX, P], bf16, tag="oh")
                    npc = len(pcs)
                    # pieces are consecutive ohcols by construction
                    oc0 = pcs[0]
                    assert pcs == list(range(oc0, oc0 + npc))
                    nc.vector.tensor_tensor(
                        out=oh[:, 0:npc, :],
                        in0=iota_t[:].unsqueeze(1).broadcast_to([P, npc, P]),
                        in1=dstrel_t[:, oc0:oc0 + npc]
                            .unsqueeze(2).broadcast_to([P, npc, P]),
                        op=OP.is_equal)
                    gf = gp.tile([P, BMAX, TABW], bf16, tag="gf")
                    nc.gpsimd.dma_gather(
                        out_ap=gf[:, 0:nblk, :],
                        in_ap=tabq[l - 1][c][:, :],
                        idxs_ap=gidx_t[:, coff:coff + rows // 16],
                        num_idxs=rows, num_idxs_reg=reg_for(rows),
                        elem_size=TABW, queue_num=qn_state[0])
                    qn_state[0] = (qn_state[0] + 1) % 4
                    gf_of[cid] = gf
                    oh_of[cid] = (oh, oc0)

                for t in range(NT):
                    for (cid, b, ohcol, lo_c, hi_c, lo_b) in pieces_by_tile[t]:
                        if cid not in gf_of:
                            emit_call(cid)
                    ps = psum.tile([P, HID], f32, tag="psB")
                    nc.tensor.matmul(
                        out=ps[:], lhsT=identb[:],
                        rhs=hp_sb[l - 1][:, t * HID:(t + 1) * HID],
                        start=True, stop=False)
                    npieces_t = len(pieces_by_tile[t])
                    for i, (cid, b, ohcol, lo_c, hi_c, lo_b) in enumerate(
                            pieces_by_tile[t]):
                        oh, oc0 = oh_of[cid]
                        nc.tensor.matmul(
                            out=ps[:],
                            lhsT=oh[:, ohcol - oc0, :],
                            rhs=gf_of[cid][:, b, 0:HID],
                            start=False,
                            stop=(i == npieces_t - 1))
                    t1 = tmp.tile([P, HID], f32, tag="t1")
                    nc.scalar.activation(out=t1[:], in_=ps[:], func=AF.Copy,
                                         scale=dis_t[:, t:t + 1])
                    nc.vector.tensor_tensor(out=t1[:], in0=t1[:], in1=b_t[:],
                                            op=OP.add)
                    if l == 1:
                        z = tmp.tile([P, HID], f32, tag="z")
                        nc.scalar.activation(out=z[:], in_=t1[:], func=AF.Relu)
                        psE = psumT.tile([HID, P], f32, tag="psE")
                        nc.tensor.transpose(out=psE[:], in_=z[:],
                                            identity=ident[:])
                        nc.scalar.copy(out=zT_sb[:, t * P:(t + 1) * P],
                                       in_=psE[:])
                        if interleave_A2:
                            tile_A(2, t, None)
                            for q in range(4):
                                if q_last_tile[q] == t:
                                    fire_AG(2, q)
                    else:
                        mzz = tmp.tile([P, HID], f32, tag="m")
                        nc.vector.tensor_tensor(out=mzz[:], in0=t1[:],
                                                in1=Wl_t[:], op=OP.mult)
                        r = tmp.tile([P, 1], f32, tag="r")
                        nc.vector.tensor_reduce(out=r[:], in_=mzz[:],
                                                axis=mybir.AxisListType.X,
                                                op=OP.add)
                        nc.scalar.activation(out=y_sb[:, t:t + 1], in_=r[:],
                                             func=AF.Sigmoid, bias=bl_t[:, 0:1])

            with tc.tile_pool(name="xt", bufs=1) as xtp:
                xT_t = xtp.tile([P, NLOC], f32, tag="xT")
                nc.sync.dma_start(out=xT_t[:], in_=xT_d[:, :])
                for t in range(NT):
                    tile_A(1, t, xT_t)
                    for q in range(4):
                        if q_last_tile[q] == t:
                            fire_AG(1, q)

            with tc.tile_pool(name="gath", bufs=20) as gp, \
                 tc.tile_pool(name="ohp", bufs=10) as ohp:
                for _w in range(20):
                    gfw = gp.tile([P, BMAX, TABW], bf16, tag="gf")
                    nc.vector.memset(gfw[:], 0.0)
                layer_B(1, gp, ohp, True)
                layer_B(2, gp, ohp, False)

            psG = psumT.tile([NT, P], f32, tag="psG")
            nc.tensor.matmul(out=psG[:], lhsT=y_sb[:, :NT], rhs=ident[:],
                             start=True, stop=True, is_transpose=True)
            og = tmp.tile([NT, P], f32, tag="og")
            nc.scalar.copy(out=og[:], in_=psG[:])
            nc.sync.dma_start(out=out_d[:, :], in_=og[:])
    nc.compile()
    return nc


def kernel(x, edge_index, W1, b1, W2, b2, Wl, bl):
    from concourse.bass_utils import run_bass_kernel_spmd
    in_maps, meta = _prep(x, edge_index, W1, b1, W2, b2, Wl, bl)
    nc = _program(meta, float(np.asarray(bl).reshape(-1)[0]))
    res = run_bass_kernel_spmd(nc, in_maps, list(range(NCORES)))
    outs = []
    for c in range(NCORES):
        o = np.asarray(res.results[c]["out"], dtype=np.float32).reshape(NLOC)
        outs.append(o[:NLOC_REAL])
    return np.concatenate(outs).reshape(N_NODES, 1).astype(np.float32)
